# revision 55
# baseline (speedup 1.0000x reference)
"""ConvNeXt block (dwconv7 -> LN -> pwconv1 -> GELU -> GRN -> pwconv2 -> residual)
for Trainium2, batch-parallel across 8 NeuronCores (2 batches per core).

Self-contained: hardcodes shapes B=16, C=512, T=2048, I=1536, K=7.
Measured 173us/invocation (reps-slope) vs ~177us f16 PE roofline
(768 big matmuls + 64 stats matmuls at 2.4GHz).

Design notes:
  - everything f16 on-chip (x shipped f16 from host, out returned f16 and
    cast to f32 on host): DVE 2x/4x perf modes, half DMA, h kept in SBUF.
  - conv = per-tap tensor_scalar products (4x mode) + tensor_tensor adds
    (2x mode) on DVE only, sliced and interleaved with the per-t-tile
    LN/mm1 pipelines; batch>0 conv emitted as 1024-wide halves sized to
    the prev batch's mm2 window, remainder interleaved between mm1 tiles.
  - LN folded as ysc = y*Abc + Bbc (A=1/sqrt(var+eps), B=-mu*A broadcast
    via K=1 matmuls); stats chain for tile t+1 emitted from a hook 3
    i-chunks into mm1(t) so its PE matmuls land mid-queue instead of
    blocking the queue head while DVE/ACT deps resolve.
  - gx squares split DVE-STT/ACT-Square; engines' in-order queues mean
    emission order is the schedule.

HW-measured dead ends (do NOT retry without new evidence; CoreSim's cost
model is wrong about all of these):
  - fp8e4 DoubleRow matmuls: sim says 0.5 cyc/row (4x f16); HW measured
    ~13x SLOWER than modeled (798us total vs 264 sim). Precision was fine
    (1.51e-2 with hi+lo weight split + x64 weight scaling vs 2e-2 gate).
  - Pool/GPSIMD tensor math (conv TSP+TT, ysq TT-mult, presum adds): sim
    models full 1.2GHz; HW measured 3.6x total regression (915us). walrus
    also rejects STT/reduce/PSUM-access on Pool outright. Keep Pool to
    DMA issues, partition_broadcast, residual adds.
  - DVE chunk-presum for LN stats (PE 8->2 matmuls/tile): +80us on HW
    (253 vs 173) - the 6 serial in-place adds sit in the latency-critical
    one-tile-ahead stats chain.
  - single-buffer conv scratch pool (pp bufs=1): ~+130us on HW from WAR
    semaphore serialization of the tap chain. Keep bufs=2.
  - ACT Sqrt forces act-table swaps (~1.4us each, ~15/invocation) since
    sqrt is not in the gelu_and_others table set; tolerated (no cheap
    alternative engine for sqrt).

Math folding (host-side, weight-sized only):
  LN:  y_ln = (y - mu_t) * A_t * ln_g + ln_b
  mm1: h_pre = w1p @ ((y - mu)*A) + b1p, w1p = w1*ln_g, b1p = b1 + w1@ln_b
  GRN: h' = h * ss[i] + grn_b[i],  ss = 1 + grn_g * gx * d,
       gx = sqrt(sum_t h^2), d = 1/(mean_i gx + eps)
  mm2: out = (w2 * ss) @ h + (b2 + w2 @ grn_b) + residual
"""
import sys

sys.path.insert(0, "/opt/trn_rl_repo")

import numpy as np
import concourse.bacc as bacc
import concourse.tile as tile
from concourse import mybir
from concourse.bass_utils import run_bass_kernel_spmd

F32 = mybir.dt.float32
F32R = mybir.dt.float32r
F16 = mybir.dt.float16
F8 = mybir.dt.float8e4
AF = mybir.ActivationFunctionType
OP = mybir.AluOpType
DR = mybir.MatmulPerfMode.DoubleRow

# mm1 in fp8 DoubleRow (2x PE) for these t-tiles: weights split hi+lo fp8
# (scaled x64 against e4m3 subnormals, rescaled in the GELU's scale arg),
# activations single-fp8. Empirically ~1.5e-2 rel err when all 4 tiles are
# fp8 vs the 2e-2 gate; () disables.
FP8_TILES = ()
W1SCALE = 64.0

B, C, T, I, K = 16, 512, 2048, 1536, 7
NCORES = 8
BPC = B // NCORES          # batches per core
CC = C // 128              # 4 c-chunks
IC = I // 128              # 12 i-chunks
TC = T // 512              # 4 t-chunks
TN = 512                   # matmul free-dim tile
LN_EPS = 1e-6
GRN_EPS = 1e-6

_CACHE = {}


def _build(trace_sim=False, reps=1):
    nc = bacc.Bacc("TRN2", target_bir_lowering=False, debug=False,
                   num_devices=NCORES)
    dram = {}

    def din(name, shape, dt=F32):
        dram[name] = nc.dram_tensor(name, shape, dt, kind="ExternalInput").ap()
        return dram[name]

    x_d = din("x", [BPC, C, T], F16)                 # per-core batches (f16!)
    w1pT_d = (din("w1pT", [C, I], F16)               # (w1*ln_g).T  [c, i]
              if len(FP8_TILES) < TC else None)
    w1h8_d = din("w1h8", [C, I], F8) if FP8_TILES else None
    w1l8_d = din("w1l8", [C, I], F8) if FP8_TILES else None
    b1p_d = din("b1p", [128, IC])                    # col-chunked b1p
    w2T_d = din("w2T", [I, C], F16)                  # w2.T  [i, c]
    b2p_d = din("b2p", [128, CC])                    # col-chunked b2 + w2@grn_b
    grng_d = din("grng", [128, IC])                  # col-chunked grn_g
    dww_d = din("dww", [128, CC, K])                 # depthwise taps per c-chunk
    dwb_d = din("dwb", [128, CC])                    # depthwise bias per c-chunk
    out_d = nc.dram_tensor("out", [BPC, C, T], F16, kind="ExternalOutput").ap()

    from contextlib import ExitStack
    with tile.TileContext(nc, trace_sim=trace_sim) as tc:
        ctx = ExitStack()
        with ctx:
            E = _setup(nc, tc, ctx, w1pT_d, b1p_d, w2T_d, b2p_d,
                       grng_d, dww_d, dwb_d, w1h8_d, w1l8_d)
            for _ in range(reps):
                _kernel_body(nc, tc, E, x_d, out_d)
    nc.compile()
    return nc


def _setup(nc, tc, ctx, w1pT_d, b1p_d, w2T_d, b2p_d, grng_d, dww_d, dwb_d,
           w1h8_d=None, w1l8_d=None):
    """Pools + weight loads, shared across reps so reps pipeline like
    batches do (per-rep pools would serialize reps on weight-tile WAR)."""
    from types import SimpleNamespace
    if True:
        ctx.enter_context(nc.allow_low_precision(
            reason="f16 on-chip compute is intentional; tolerance is 2e-2"))
        singles = ctx.enter_context(tc.tile_pool(name="singles", bufs=1))
        xp = ctx.enter_context(tc.tile_pool(name="xp", bufs=2))
        yp = ctx.enter_context(tc.tile_pool(name="yp", bufs=2))
        pp = ctx.enter_context(tc.tile_pool(name="pp", bufs=2))
        ysqp = ctx.enter_context(tc.tile_pool(name="ysqp", bufs=4))
        sump = ctx.enter_context(tc.tile_pool(name="sump", bufs=1))
        rowp = ctx.enter_context(tc.tile_pool(name="rowp", bufs=4))
        abp = ctx.enter_context(tc.tile_pool(name="abp", bufs=2))
        yscp = ctx.enter_context(tc.tile_pool(name="yscp", bufs=2))
        hp = ctx.enter_context(tc.tile_pool(name="hp", bufs=4))
        sqp = ctx.enter_context(tc.tile_pool(name="sqp", bufs=2))
        gxp = ctx.enter_context(tc.tile_pool(name="gxp", bufs=2))
        w2p = ctx.enter_context(tc.tile_pool(name="w2p", bufs=1))
        w2sp = ctx.enter_context(tc.tile_pool(name="w2sp", bufs=1))
        op_ = ctx.enter_context(tc.tile_pool(name="op", bufs=2))
        # PSUM pools: 2 ph + 2 po + 2 stats + 2 (one [128,1024] AB) = 8 banks
        mmps = ctx.enter_context(tc.tile_pool(name="mmps", bufs=2, space="PSUM"))
        pops = ctx.enter_context(tc.tile_pool(name="pops", bufs=2, space="PSUM"))
        smps = ctx.enter_context(tc.tile_pool(name="smps", bufs=4, space="PSUM"))

        # ---- tiny constants (no DMA deps) ----
        invc_f = singles.tile([128, 1], F32)
        nc.vector.memset(invc_f[:], 1.0 / C)
        ones_col = singles.tile([128, 1], F16)    # stats lhsT: gives mean directly
        nc.vector.tensor_copy(ones_col[:], invc_f[:])
        ones1_f = singles.tile([128, 1], F32)
        nc.vector.memset(ones1_f[:], 1.0)
        ones_col1 = singles.tile([128, 1], F32R)  # gsum lhsT (K=128, M=1)
        nc.vector.tensor_copy(ones_col1[:], ones1_f[:])
        eps_ln = singles.tile([1, 1], F32)
        nc.vector.memset(eps_ln[:], LN_EPS)

        # ---- constants ----
        dww = singles.tile([128, CC, K], F32)
        nc.gpsimd.dma_start(dww[:], dww_d)
        dwb = singles.tile([128, CC], F32)
        nc.gpsimd.dma_start(dwb[:], dwb_d)
        b1p = singles.tile([128, IC], F32)
        nc.gpsimd.dma_start(b1p[:], b1p_d)
        b2p = singles.tile([128, CC], F32)
        nc.gpsimd.dma_start(b2p[:], b2p_d)
        grng = singles.tile([128, IC], F32)
        nc.gpsimd.dma_start(grng[:], grng_d)
        w1pT = None
        if len(FP8_TILES) < TC:   # f16 weights only needed for f16 tiles
            w1pT = singles.tile([128, CC, I], F16)
            nc.sync.dma_start(w1pT[:],
                              w1pT_d.rearrange("(cc p) i -> p cc i", p=128))
        w2t = w2p.tile([128, IC, C], F16)
        nc.sync.dma_start(w2t[:], w2T_d.rearrange("(ic p) c -> p ic c", p=128))
        w1h8 = w1l8 = None
        if w1h8_d is not None:
            w1h8 = singles.tile([128, CC, I], F8)
            nc.scalar.dma_start(w1h8[:],
                                w1h8_d.rearrange("(cc p) i -> p cc i", p=128))
            w1l8 = singles.tile([128, CC, I], F8)
            nc.scalar.dma_start(w1l8[:],
                                w1l8_d.rearrange("(cc p) i -> p cc i", p=128))

        return SimpleNamespace(**{k: v for k, v in locals().items()
                                  if k not in ("nc", "tc", "ctx")})


def _kernel_body(nc, tc, E, x_d, out_d):
    if True:
        (singles, xp, yp, pp, ysqp, sump, rowp, abp, yscp, hp, sqp,
         gxp, w2p, w2sp, op_, mmps, pops, smps) = (
            E.singles, E.xp, E.yp, E.pp, E.ysqp, E.sump, E.rowp,
            E.abp, E.yscp, E.hp, E.sqp, E.gxp, E.w2p, E.w2sp, E.op_,
            E.mmps, E.pops, E.smps)
        (ones_col, ones_col1, eps_ln, dww, dwb, b1p, b2p, grng, w1pT,
         w2t, w1h8, w1l8) = (E.ones_col, E.ones_col1, E.eps_ln, E.dww,
                             E.dwb, E.b1p, E.b2p, E.grng, E.w1pT, E.w2t,
                             E.w1h8, E.w1l8)
        xv = x_d.rearrange("b (cc p) t -> b p cc t", p=128)

        # ---- per-batch persistent tiles ----
        x_tiles = []
        for b in range(BPC):
            x_t = xp.tile([128, CC, T], F16, tag="x")
            for ci in range(CC):
                nc.gpsimd.dma_start(x_t[:, ci, :], xv[b, :, ci, :])
            x_tiles.append(x_t)

        y_tiles = {}
        gxparts = {}
        h_tiles = {}

        def emit_conv_slice(b, a0, a1, pool_cc=(), cc_list=None):
            """Depthwise conv over [a0, a1): TSP products (4x) + TT adds (2x).
            Chunks in pool_cc run as stt chains on GPSIMD (startup balance).
            cc_list limits which chunks are emitted (for split emission)."""
            x_t = x_tiles[b]
            y_t = y_tiles[b]
            n = a1 - a0
            for ci in (range(CC) if cc_list is None else cc_list):
                acc = y_t[:, ci, a0:a1]
                pool = ci in pool_cc
                eng = nc.gpsimd if pool else nc.vector
                eng.tensor_scalar(acc, x_t[:, ci, a0:a1],
                                  dww[:, ci, 3:4], dwb[:, ci:ci + 1],
                                  OP.mult, OP.add)
                for k in (0, 1, 2, 4, 5, 6):
                    d = k - 3
                    lo = max(a0, -d)
                    hi = min(a1, T - d)
                    m = hi - lo
                    # walrus rejects STT on Pool: TSP product + TT add on
                    # both engines (Pool pays 2 passes/tap)
                    p_k = pp.tile([128, n], F16, tag="p",
                                  padded_shape=[128, 1024])
                    eng.tensor_scalar(p_k[:, 0:m],
                                      x_t[:, ci, lo + d:hi + d],
                                      dww[:, ci, k:k + 1], None,
                                      OP.mult)
                    eng.tensor_add(acc[:, lo - a0:hi - a0],
                                   acc[:, lo - a0:hi - a0],
                                   p_k[:, 0:m])

        ysq_tiles = {}

        def emit_ysq(b, t, eng="act"):
            y_t = y_tiles[b]
            ts_ = slice(t * TN, (t + 1) * TN)
            ysq = ysqp.tile([128, CC, TN], F16, tag="ysq")
            if eng == "act":
                nc.scalar.activation(ysq[:], y_t[:, :, ts_], AF.Square)
            elif eng == "pool":
                nc.vector.tensor_mul(ysq[:], y_t[:, :, ts_], y_t[:, :, ts_])
            else:  # DVE TT-mult (f16 2x) - used for batch 0 so ACT's weight
                # DMA issues don't sit ahead of the startup stats chain
                nc.vector.tensor_mul(ysq[:], y_t[:, :, ts_], y_t[:, :, ts_])
            ysq_tiles[(b, t)] = ysq

        def emit_tile_stats(b, t, sum_eng="pool"):
            """LN stats + A/B rows + broadcast + ysc for t-tile. Returns ysc.
            Chunk pre-sums cut the PE stats matmuls 8 -> 2; they run on Pool
            (DVE for the b0 prologue where Pool is conv-loaded)."""
            y_t = y_tiles[b]
            ts_ = slice(t * TN, (t + 1) * TN)
            ysq = ysq_tiles.pop((b, t))
            # tree-shaped chunk pre-sums on DVE (distinct output slots, no
            # in-place RAW chains) cut PE stats matmuls 8 -> 2 per tile;
            # the earlier serial in-place variant measured +80us on HW
            s_ = sump.tile([128, 4, TN], F16, tag="sum")
            nc.vector.tensor_add(s_[:, 0, :], y_t[:, 0, ts_], y_t[:, 1, ts_])
            nc.vector.tensor_add(s_[:, 1, :], y_t[:, 2, ts_], y_t[:, 3, ts_])
            nc.vector.tensor_add(s_[:, 2, :], s_[:, 0, :], s_[:, 1, :])
            nc.vector.tensor_add(s_[:, 0, :], ysq[:, 0, :], ysq[:, 1, :])
            nc.vector.tensor_add(s_[:, 1, :], ysq[:, 2, :], ysq[:, 3, :])
            nc.vector.tensor_add(s_[:, 3, :], s_[:, 0, :], s_[:, 1, :])
            mu_ps = smps.tile([1, TN], F32, tag="sm")
            ey2_ps = smps.tile([1, TN], F32, tag="sm")
            nc.tensor.matmul(mu_ps[:], ones_col[:], s_[:, 2, :],
                             start=True, stop=True)
            nc.tensor.matmul(ey2_ps[:], ones_col[:], s_[:, 3, :],
                             start=True, stop=True)
            mu_sb = rowp.tile([1, TN], F32, tag="r")
            nc.vector.tensor_copy(mu_sb[:], mu_ps[:])
            msq = rowp.tile([1, TN], F32, tag="r")
            nc.vector.tensor_mul(msq[:], mu_sb[:], mu_sb[:])
            var = rowp.tile([1, TN], F32, tag="r")
            nc.vector.tensor_sub(var[:], ey2_ps[:], msq[:])
            stdv = rowp.tile([1, TN], F32, tag="r")
            nc.scalar.activation(stdv[:], var[:], AF.Sqrt, bias=eps_ln[:])
            ab_row = rowp.tile([1, 2 * TN], F16, tag="ab")
            nc.vector.reciprocal(ab_row[:, 0:TN], stdv[:])
            # B = -mu * A
            nc.vector.scalar_tensor_tensor(ab_row[:, TN:2 * TN], mu_sb[:], -1.0,
                                           ab_row[:, 0:TN], OP.mult, OP.mult)
            # broadcast A,B across partitions on GPSIMD (off the PE stream;
            # the one-tile-ahead chain slack absorbs the Q7 latency)
            ab_sb = abp.tile([128, 2 * TN], F16, tag="absb")
            nc.gpsimd.partition_broadcast(ab_sb[:], ab_row[:])
            # ysc = y*A + B
            if t in FP8_TILES:
                # fp8 rhs for the DoubleRow mm1: mul to f16 temp (2x), add
                # writes f8 (1x)
                ysc = yscp.tile([128, CC, TN], F8, tag="ysc8")
                for ci in range(CC):
                    ytmp = ytp.tile([128, TN], F16, tag="yt")
                    nc.vector.tensor_mul(ytmp[:], y_t[:, ci, ts_],
                                         ab_sb[:, 0:TN])
                    nc.vector.tensor_add(ysc[:, ci, :], ytmp[:],
                                         ab_sb[:, TN:2 * TN])
                return ysc
            ysc = yscp.tile([128, CC, TN], F16, tag="ysc")
            for ci in range(CC):
                nc.vector.tensor_mul(ysc[:, ci, :], y_t[:, ci, ts_],
                                     ab_sb[:, 0:TN])
                nc.vector.tensor_add(ysc[:, ci, :], ysc[:, ci, :],
                                     ab_sb[:, TN:2 * TN])
            return ysc

        def emit_tile_mm1(b, t, ysc, mid=None, sq_dve=8):
            """mid() is invoked a few i-chunks in: the next tile's stats
            chain emitted there lands mid-PE-queue, so its matmuls don't
            block the queue head while the DVE pre-sums run."""
            gxpart = gxparts[b]
            h_t = hp.tile([128, IC, TN], F16, tag="h")
            h_tiles[(b, t)] = h_t
            mid_out = [None]
            fp8 = t in FP8_TILES
            for ii in range(IC):
                if ii == 3 and mid is not None:
                    mid_out[0] = mid()
                ph = mmps.tile([128, TN], F32, tag="mm")
                isl = slice(ii * 128, (ii + 1) * 128)
                if fp8:
                    # DoubleRow fp8: each instr contracts a 256-row chunk
                    # pair; hi then lo weight terms accumulate in PSUM
                    for wi, w8 in enumerate((w1h8, w1l8)):
                        for cp in range(CC // 2):
                            nc.tensor.matmul(
                                ph[:], w8[:, 2 * cp:2 * cp + 2, isl],
                                ysc[:, 2 * cp:2 * cp + 2, :], perf_mode=DR,
                                start=(wi == 0 and cp == 0),
                                stop=(wi == 1 and cp == CC // 2 - 1))
                    nc.scalar.activation(h_t[:, ii, :], ph[:], AF.Gelu,
                                         bias=b1p[:, ii:ii + 1],
                                         scale=1.0 / W1SCALE)
                else:
                    for ci in range(CC):
                        nc.tensor.matmul(ph[:], w1pT[:, ci, isl],
                                         ysc[:, ci, :],
                                         start=(ci == 0), stop=(ci == CC - 1))
                    nc.scalar.activation(h_t[:, ii, :], ph[:], AF.Gelu,
                                         bias=b1p[:, ii:ii + 1])
                sq = sqp.tile([128, TN], F16, tag="sq")
                # h-squares: baseline-proven split DVE-STT / ACT-Square
                if (ii in (0, 3, 5, 8, 10)) if sq_dve == 4 else (ii % 3 != 2):
                    nc.vector.scalar_tensor_tensor(
                        sq[:], h_t[:, ii, :], 1.0, h_t[:, ii, :],
                        OP.bypass, OP.mult,
                        accum_out=gxpart[:, ii, t:t + 1])
                else:
                    nc.scalar.activation(
                        sq[:], h_t[:, ii, :], AF.Square,
                        accum_out=gxpart[:, ii, t:t + 1])
            return mid_out[0]

        def emit_grn(b):
            """GRN scale factors -> scaled w2 copy."""
            gxpart = gxparts[b]
            gxsq = gxp.tile([128, IC], F32, tag="gx2")
            nc.vector.tensor_reduce(gxsq[:], gxpart[:],
                                    axis=mybir.AxisListType.X, op=OP.add)
            gx = gxp.tile([128, IC], F32R, tag="gx2")
            nc.scalar.activation(gx[:], gxsq[:], AF.Sqrt)
            gsum = smps.tile([1, IC], F32, tag="sm")
            nc.tensor.matmul(gsum[:], ones_col1[:], gx[:], start=True, stop=True)
            gtot = gxp.tile([1, 1], F32, tag="gx3")
            nc.vector.tensor_reduce(gtot[:], gsum[:], axis=mybir.AxisListType.X,
                                    op=OP.add)
            dinv = gxp.tile([1, 1], F32, tag="gx3")
            nc.vector.tensor_scalar(dinv[:], gtot[:], 1.0 / I, GRN_EPS,
                                    OP.mult, OP.add)
            d_row = gxp.tile([1, 1], F32R, tag="gx3")
            nc.vector.reciprocal(d_row[:], dinv[:])
            dbc = gxp.tile([128, 1], F32, tag="gx4")
            nc.gpsimd.partition_broadcast(dbc[:], d_row[:].bitcast(F32))
            ss = gxp.tile([128, IC], F32, tag="gx4")
            nc.vector.scalar_tensor_tensor(ss[:], gx[:].bitcast(F32), dbc[:],
                                           grng[:], OP.mult, OP.mult)
            nc.vector.tensor_scalar(ss[:], ss[:], 1.0, None, OP.add)
            w2sc = w2sp.tile([128, IC, C], F16, tag="w2sc")
            for ii in range(IC):
                nc.vector.tensor_scalar(w2sc[:, ii, :], w2t[:, ii, :],
                                        ss[:, ii:ii + 1], None, OP.mult)
            return w2sc

        def emit_mm2(b, w2sc, prefetch=None):
            """mm2 + bias + residual + store. prefetch() emitted mid-stream."""
            x_t = x_tiles[b]
            group = 0
            for t in range(TC):
                ts_ = slice(t * TN, (t + 1) * TN)
                h_t = h_tiles[(b, t)]
                for ci in range(CC):
                    csl = slice(ci * 128, (ci + 1) * 128)
                    po = pops.tile([128, TN], F32, tag="po")
                    for ii in range(IC):
                        nc.tensor.matmul(po[:], w2sc[:, ii, csl],
                                         h_t[:, ii, :],
                                         start=(ii == 0), stop=(ii == IC - 1))
                    o_sb = op_.tile([128, TN], F16)
                    # GPSIMD cannot read PSUM (walrus-verified): evac+bias
                    # on ACT, residual add on Pool; f16 out halves the DMA
                    nc.scalar.activation(o_sb[:], po[:], AF.Identity,
                                         bias=b2p[:, ci:ci + 1])
                    nc.gpsimd.tensor_add(o_sb[:], o_sb[:], x_t[:, ci, ts_])
                    nc.gpsimd.dma_start(
                        out_d[b, ci * 128:(ci + 1) * 128, ts_], o_sb[:])
                    group += 1
                    if group == 10 and prefetch is not None:
                        prefetch()

        # ================= schedule =================
        # stats chains run one t-tile ahead of mm1 so the LN row-math /
        # broadcast latency hides under the previous tile's matmuls; ysq
        # ops are emitted right after conv so they don't queue behind GELUs.
        CONV_SLICES = [(0, 512), (512, 1024), (1024, 2048)]
        # conv chunk split: Pool takes chunks {2,3} of batch 0 (runs at
        # kernel start / early mm1 windows when Pool is light); slice 2 is
        # emitted in two halves between stats chains so ysc(t+1) on DVE
        # isn't queued behind the whole tail of the conv.
        BN_POOL = [(3,), (3,), ()]
        prefetched_ysc = {}

        for b in range(BPC):
            if b not in y_tiles:
                y_tiles[b] = yp.tile([128, CC, T], F16, tag="y", name=f"y{b}")
                gxparts[b] = gxp.tile([128, IC, TC], F32, tag="gxpart",
                                      name=f"gxp{b}")
            if b in prefetched_ysc:
                ysc_prev = prefetched_ysc.pop(b)
                ysc_prev = emit_tile_mm1(b, 0, ysc_prev, sq_dve=4,
                                         mid=lambda b=b: emit_tile_stats(b, 1))
                # rest of this batch's conv (chunks 2/3, t2/t3 range) in
                # 512-quarters interleaved between mm1 tiles so each stats
                # chain's data lands just before its mid-hook fires
                emit_conv_slice(b, 1024, 1536, cc_list=[2, 3])
                emit_ysq(b, 2, eng="dve")
                ysc_prev = emit_tile_mm1(b, 1, ysc_prev, sq_dve=4,
                                         mid=lambda b=b: emit_tile_stats(b, 2))
                emit_conv_slice(b, 1536, 2048, cc_list=[2, 3])
                emit_ysq(b, 3, eng="dve")
                ysc_prev = emit_tile_mm1(b, 2, ysc_prev, sq_dve=4,
                                         mid=lambda b=b: emit_tile_stats(b, 3))
                emit_tile_mm1(b, 3, ysc_prev, sq_dve=4)
            else:
                emit_conv_slice(b, *CONV_SLICES[0])
                emit_ysq(b, 0, eng="dve")
                ysc0 = emit_tile_stats(b, 0, sum_eng="dve")
                emit_conv_slice(b, *CONV_SLICES[1])
                emit_ysq(b, 1, eng="dve")
                ysc1 = emit_tile_mm1(b, 0, ysc0, sq_dve=4,
                                     mid=lambda b=b: emit_tile_stats(b, 1))
                emit_conv_slice(b, 1024, 1536)
                emit_ysq(b, 2, eng="dve")
                ysc2 = emit_tile_mm1(b, 1, ysc1, sq_dve=4,
                                     mid=lambda b=b: emit_tile_stats(b, 2))
                emit_conv_slice(b, 1536, 2048)
                emit_ysq(b, 3, eng="dve")
                ysc3 = emit_tile_mm1(b, 2, ysc2, sq_dve=4,
                                     mid=lambda b=b: emit_tile_stats(b, 3))
                emit_tile_mm1(b, 3, ysc3, sq_dve=4)
            w2sc = emit_grn(b)
            prefetch = None
            if b + 1 < BPC:
                nb = b + 1
                y_tiles[nb] = yp.tile([128, CC, T], F16, tag="y", name=f"y{nb}")
                gxparts[nb] = gxp.tile([128, IC, TC], F32, tag="gxpart",
                                       name=f"gxp{nb}")
                # mm2-window conv: chunks 0-1 fully + chunk 2 first half on
                # DVE (~32us incl w2sc + prefetched stats), chunk 3 first
                # half on Pool; second halves are emitted after mm1(nb, t0).
                emit_conv_slice(nb, 0, 1024, cc_list=[0, 1])
                emit_conv_slice(nb, 1024, 2048, cc_list=[0, 1])
                emit_conv_slice(nb, 0, 1024, cc_list=[2, 3])

                def prefetch(nb=nb):
                    # ysq emitted here (mid-mm2 stream) so the ACT queue
                    # drains batch b's po groups before b+1's squares
                    emit_ysq(nb, 0)
                    prefetched_ysc[nb] = emit_tile_stats(nb, 0)
                    emit_ysq(nb, 1)
            emit_mm2(b, w2sc, prefetch=prefetch)


def _host_prep(inputs):
    w1 = inputs["w1"].astype(np.float64)
    ln_g = inputs["ln_g"].astype(np.float64)
    ln_b = inputs["ln_b"].astype(np.float64)
    w2 = inputs["w2"].astype(np.float64)
    w1p = w1 * ln_g[None, :]                         # [I, C]
    prep = {
        "b1p": (inputs["b1"].astype(np.float64) + w1 @ ln_b)
               .astype(np.float32).reshape(IC, 128).T.copy(),
        "w2T": np.ascontiguousarray(w2.T).astype(np.float16),
        "b2p": (inputs["b2"].astype(np.float64)
                + w2 @ inputs["grn_b"].astype(np.float64))
               .astype(np.float32).reshape(CC, 128).T.copy(),
        "grng": inputs["grn_g"].reshape(IC, 128).T.copy().astype(np.float32),
        "dww": inputs["dw_w"].reshape(C, K).reshape(CC, 128, K)
               .transpose(1, 0, 2).copy().astype(np.float32),
        "dwb": inputs["dw_b"].reshape(CC, 128).T.copy().astype(np.float32),
    }
    if len(FP8_TILES) < TC:
        prep["w1pT"] = np.ascontiguousarray(w1p.T).astype(np.float16)
    if FP8_TILES:
        f8np = mybir.dt.np(F8)
        w1ps = np.ascontiguousarray(w1p.T) * W1SCALE          # [C, I] f64
        w1h = w1ps.astype(np.float32).astype(f8np)
        w1l = (w1ps - w1h.astype(np.float64)).astype(np.float32).astype(f8np)
        prep["w1h8"] = w1h
        prep["w1l8"] = w1l
    return prep


def run(inputs, trace=False, **kw):
    if "nc" not in _CACHE:
        _CACHE["nc"] = _build()
    nc = _CACHE["nc"]
    prep = _host_prep(inputs)
    x = np.asarray(inputs["x"], dtype=np.float32).astype(np.float16)
    in_maps = []
    for c in range(NCORES):
        m = dict(prep)
        m["x"] = np.ascontiguousarray(x[c * BPC:(c + 1) * BPC])
        in_maps.append(m)
    res = run_bass_kernel_spmd(nc, in_maps, core_ids=list(range(NCORES)),
                               trace=trace, **kw)
    out = np.concatenate([r["out"] for r in res.results], axis=0)
    return out.astype(np.float32), res


def kernel(**inputs):
    out, _ = run(inputs)
    return out



# revision 57
# speedup vs baseline: 1.1124x; 1.1124x over previous
"""ConvNeXt block (dwconv7 -> LN -> pwconv1 -> GELU -> GRN -> pwconv2 -> residual)
for Trainium2, batch-parallel across 8 NeuronCores (2 batches per core).

Self-contained: hardcodes shapes B=16, C=512, T=2048, I=1536, K=7.
Measured 173us/invocation (reps-slope) at the f16 PE roofline
(768 big matmuls + 64 stats matmuls ~ 177us at 2.4GHz).

Design notes:
  - everything f16 on-chip (x shipped f16 from host, out returned f16,
    cast to f32 on host): DVE 2x/4x perf modes, half DMA, h kept in SBUF.
  - conv = per-tap tensor_scalar products (4x mode) + tensor_tensor adds
    (2x mode) on DVE only, sliced and interleaved with the per-t-tile
    LN/mm1 pipelines; batch>0 conv emitted as 1024-wide halves sized to
    the prev batch's mm2 window, remainder interleaved between mm1 tiles.
  - LN folded as ysc = y*Abc + Bbc (A=1/sqrt(var+eps), B=-mu*A broadcast
    via K=1 matmuls); the stats chain for tile t+1 is emitted from a hook
    3 i-chunks into mm1(t) so its PE matmuls land mid-queue instead of
    blocking the PE queue head while DVE/ACT deps resolve.
  - gx squares split DVE-STT/ACT-Square; engines' in-order queues mean
    emission order is the schedule.

HW-measured dead ends (do NOT retry without new evidence; CoreSim's cost
model is wrong about all of these):
  - fp8e4 DoubleRow matmuls: sim says 0.5 cyc/row (4x f16); HW measured
    ~13x slower than modeled (798us total vs 264 sim). Precision was fine
    (1.51e-2 with hi+lo weight split + x64 weight scaling vs 2e-2 gate).
  - Pool/GPSIMD tensor math (conv TSP+TT, ysq TT-mult, presum adds): sim
    models full 1.2GHz; HW measured 3.6x total regression (915us). walrus
    also rejects STT/reduce/PSUM-access on Pool. Keep Pool to DMA issues,
    partition_broadcast, and residual adds.
  - DVE chunk-presum for LN stats (PE 8->2 matmuls/tile, -10us PE): BOTH
    the serial in-place form (253us) and the tree form with distinct
    slots (272us) regress vs 173us - DVE has no headroom on HW and the
    adds sit in the latency-critical one-tile-ahead stats chain.
  - single-buffer conv scratch pool (pp bufs=1): ~+130us on HW from WAR
    semaphore serialization of the tap chain. Keep bufs=2.
  - ACT Sqrt forces act-table swaps (~1.4us each, ~15/invocation) since
    sqrt is not in the gelu_and_others table set; tolerated (no cheap
    alternative engine for sqrt).

Math folding (host-side, weight-sized only):
  LN:  y_ln = (y - mu_t) * A_t * ln_g + ln_b
  mm1: h_pre = w1p @ ((y - mu)*A) + b1p, w1p = w1*ln_g, b1p = b1 + w1@ln_b
  GRN: h' = h * ss[i] + grn_b[i],  ss = 1 + grn_g * gx * d,
       gx = sqrt(sum_t h^2), d = 1/(mean_i gx + eps)
  mm2: out = (w2 * ss) @ h + (b2 + w2 @ grn_b) + residual
"""
import sys

sys.path.insert(0, "/opt/trn_rl_repo")

import numpy as np
import concourse.bacc as bacc
import concourse.tile as tile
from concourse import mybir
from concourse.bass_utils import run_bass_kernel_spmd

F32 = mybir.dt.float32
F32R = mybir.dt.float32r
F16 = mybir.dt.float16
F8 = mybir.dt.float8e4
AF = mybir.ActivationFunctionType
OP = mybir.AluOpType
DR = mybir.MatmulPerfMode.DoubleRow

# mm1 in fp8 DoubleRow (2x PE) for these t-tiles: weights split hi+lo fp8
# (scaled x64 against e4m3 subnormals, rescaled in the GELU's scale arg),
# activations single-fp8. Empirically ~1.5e-2 rel err when all 4 tiles are
# fp8 vs the 2e-2 gate; () disables.
FP8_TILES = ()
W1SCALE = 64.0

B, C, T, I, K = 16, 512, 2048, 1536, 7
NCORES = 8
BPC = B // NCORES          # batches per core
CC = C // 128              # 4 c-chunks
IC = I // 128              # 12 i-chunks
TC = T // 512              # 4 t-chunks
TN = 512                   # matmul free-dim tile
LN_EPS = 1e-6
GRN_EPS = 1e-6

_CACHE = {}


def _build(trace_sim=False, reps=1):
    nc = bacc.Bacc("TRN2", target_bir_lowering=False, debug=False,
                   num_devices=NCORES)
    dram = {}

    def din(name, shape, dt=F32):
        dram[name] = nc.dram_tensor(name, shape, dt, kind="ExternalInput").ap()
        return dram[name]

    x_d = din("x", [BPC, C, T], F16)                 # per-core batches (f16!)
    w1pT_d = (din("w1pT", [C, I], F16)               # (w1*ln_g).T  [c, i]
              if len(FP8_TILES) < TC else None)
    w1h8_d = din("w1h8", [C, I], F8) if FP8_TILES else None
    w1l8_d = din("w1l8", [C, I], F8) if FP8_TILES else None
    b1p_d = din("b1p", [128, IC])                    # col-chunked b1p
    w2T_d = din("w2T", [I, C], F16)                  # w2.T  [i, c]
    b2p_d = din("b2p", [128, CC])                    # col-chunked b2 + w2@grn_b
    grng_d = din("grng", [128, IC])                  # col-chunked grn_g
    dww_d = din("dww", [128, CC, K])                 # depthwise taps per c-chunk
    dwb_d = din("dwb", [128, CC])                    # depthwise bias per c-chunk
    out_d = nc.dram_tensor("out", [BPC, C, T], F16, kind="ExternalOutput").ap()

    from contextlib import ExitStack
    with tile.TileContext(nc, trace_sim=trace_sim) as tc:
        ctx = ExitStack()
        with ctx:
            E = _setup(nc, tc, ctx, w1pT_d, b1p_d, w2T_d, b2p_d,
                       grng_d, dww_d, dwb_d, w1h8_d, w1l8_d)
            for _ in range(reps):
                _kernel_body(nc, tc, E, x_d, out_d)
    nc.compile()
    return nc


def _setup(nc, tc, ctx, w1pT_d, b1p_d, w2T_d, b2p_d, grng_d, dww_d, dwb_d,
           w1h8_d=None, w1l8_d=None):
    """Pools + weight loads, shared across reps so reps pipeline like
    batches do (per-rep pools would serialize reps on weight-tile WAR)."""
    from types import SimpleNamespace
    if True:
        ctx.enter_context(nc.allow_low_precision(
            reason="f16 on-chip compute is intentional; tolerance is 2e-2"))
        singles = ctx.enter_context(tc.tile_pool(name="singles", bufs=1))
        xp = ctx.enter_context(tc.tile_pool(name="xp", bufs=2))
        yp = ctx.enter_context(tc.tile_pool(name="yp", bufs=2))
        pp = ctx.enter_context(tc.tile_pool(name="pp", bufs=2))
        ysqp = ctx.enter_context(tc.tile_pool(name="ysqp", bufs=4))
        sump = ctx.enter_context(tc.tile_pool(name="sump", bufs=2))
        rowp = ctx.enter_context(tc.tile_pool(name="rowp", bufs=4))
        abp = ctx.enter_context(tc.tile_pool(name="abp", bufs=2))
        yscp = ctx.enter_context(tc.tile_pool(name="yscp", bufs=2))
        hp = ctx.enter_context(tc.tile_pool(name="hp", bufs=4))
        sqp = ctx.enter_context(tc.tile_pool(name="sqp", bufs=2))
        gxp = ctx.enter_context(tc.tile_pool(name="gxp", bufs=2))
        w2p = ctx.enter_context(tc.tile_pool(name="w2p", bufs=1))
        w2sp = ctx.enter_context(tc.tile_pool(name="w2sp", bufs=1))
        op_ = ctx.enter_context(tc.tile_pool(name="op", bufs=2))
        # PSUM pools: 2 ph + 2 po + 2 stats + 2 (one [128,1024] AB) = 8 banks
        mmps = ctx.enter_context(tc.tile_pool(name="mmps", bufs=2, space="PSUM"))
        pops = ctx.enter_context(tc.tile_pool(name="pops", bufs=2, space="PSUM"))
        smps = ctx.enter_context(tc.tile_pool(name="smps", bufs=4, space="PSUM"))

        # ---- tiny constants (no DMA deps) ----
        invc_f = singles.tile([128, 1], F32)
        nc.vector.memset(invc_f[:], 1.0 / C)
        ones_col = singles.tile([128, 1], F16)    # stats lhsT: gives mean directly
        nc.vector.tensor_copy(ones_col[:], invc_f[:])
        ones1_f = singles.tile([128, 1], F32)
        nc.vector.memset(ones1_f[:], 1.0)
        ones_col1 = singles.tile([128, 1], F32R)  # gsum lhsT (K=128, M=1)
        nc.vector.tensor_copy(ones_col1[:], ones1_f[:])
        eps_ln = singles.tile([1, 1], F32)
        nc.vector.memset(eps_ln[:], LN_EPS)

        # ---- constants ----
        dww = singles.tile([128, CC, K], F32)
        nc.gpsimd.dma_start(dww[:], dww_d)
        dwb = singles.tile([128, CC], F32)
        nc.gpsimd.dma_start(dwb[:], dwb_d)
        b1p = singles.tile([128, IC], F32)
        nc.gpsimd.dma_start(b1p[:], b1p_d)
        b2p = singles.tile([128, CC], F32)
        nc.gpsimd.dma_start(b2p[:], b2p_d)
        grng = singles.tile([128, IC], F32)
        nc.gpsimd.dma_start(grng[:], grng_d)
        w1pT = None
        if len(FP8_TILES) < TC:   # f16 weights only needed for f16 tiles
            w1pT = singles.tile([128, CC, I], F16)
            nc.sync.dma_start(w1pT[:],
                              w1pT_d.rearrange("(cc p) i -> p cc i", p=128))
        w2t = w2p.tile([128, IC, C], F16)
        nc.sync.dma_start(w2t[:], w2T_d.rearrange("(ic p) c -> p ic c", p=128))
        w1h8 = w1l8 = None
        if w1h8_d is not None:
            w1h8 = singles.tile([128, CC, I], F8)
            nc.scalar.dma_start(w1h8[:],
                                w1h8_d.rearrange("(cc p) i -> p cc i", p=128))
            w1l8 = singles.tile([128, CC, I], F8)
            nc.scalar.dma_start(w1l8[:],
                                w1l8_d.rearrange("(cc p) i -> p cc i", p=128))

        return SimpleNamespace(**{k: v for k, v in locals().items()
                                  if k not in ("nc", "tc", "ctx")})


def _kernel_body(nc, tc, E, x_d, out_d):
    if True:
        (singles, xp, yp, pp, ysqp, sump, rowp, abp, yscp, hp, sqp,
         gxp, w2p, w2sp, op_, mmps, pops, smps) = (
            E.singles, E.xp, E.yp, E.pp, E.ysqp, E.sump, E.rowp,
            E.abp, E.yscp, E.hp, E.sqp, E.gxp, E.w2p, E.w2sp, E.op_,
            E.mmps, E.pops, E.smps)
        (ones_col, ones_col1, eps_ln, dww, dwb, b1p, b2p, grng, w1pT,
         w2t, w1h8, w1l8) = (E.ones_col, E.ones_col1, E.eps_ln, E.dww,
                             E.dwb, E.b1p, E.b2p, E.grng, E.w1pT, E.w2t,
                             E.w1h8, E.w1l8)
        xv = x_d.rearrange("b (cc p) t -> b p cc t", p=128)

        # ---- per-batch persistent tiles ----
        x_tiles = []
        for b in range(BPC):
            x_t = xp.tile([128, CC, T], F16, tag="x")
            for ci in range(CC):
                nc.gpsimd.dma_start(x_t[:, ci, :], xv[b, :, ci, :])
            x_tiles.append(x_t)

        y_tiles = {}
        gxparts = {}
        h_tiles = {}

        def emit_conv_slice(b, a0, a1, pool_cc=(), cc_list=None):
            """Depthwise conv over [a0, a1): TSP products (4x) + TT adds (2x).
            Chunks in pool_cc run as stt chains on GPSIMD (startup balance).
            cc_list limits which chunks are emitted (for split emission)."""
            x_t = x_tiles[b]
            y_t = y_tiles[b]
            n = a1 - a0
            for ci in (range(CC) if cc_list is None else cc_list):
                acc = y_t[:, ci, a0:a1]
                pool = ci in pool_cc
                eng = nc.gpsimd if pool else nc.vector
                eng.tensor_scalar(acc, x_t[:, ci, a0:a1],
                                  dww[:, ci, 3:4], dwb[:, ci:ci + 1],
                                  OP.mult, OP.add)
                for k in (0, 1, 2, 4, 5, 6):
                    d = k - 3
                    lo = max(a0, -d)
                    hi = min(a1, T - d)
                    m = hi - lo
                    # walrus rejects STT on Pool: TSP product + TT add on
                    # both engines (Pool pays 2 passes/tap)
                    p_k = pp.tile([128, n], F16, tag="p",
                                  padded_shape=[128, 1024])
                    eng.tensor_scalar(p_k[:, 0:m],
                                      x_t[:, ci, lo + d:hi + d],
                                      dww[:, ci, k:k + 1], None,
                                      OP.mult)
                    eng.tensor_add(acc[:, lo - a0:hi - a0],
                                   acc[:, lo - a0:hi - a0],
                                   p_k[:, 0:m])

        ysq_tiles = {}

        def emit_ysq(b, t, eng="act"):
            y_t = y_tiles[b]
            ts_ = slice(t * TN, (t + 1) * TN)
            ysq = ysqp.tile([128, CC, TN], F16, tag="ysq")
            if eng == "act":
                nc.scalar.activation(ysq[:], y_t[:, :, ts_], AF.Square)
            elif eng == "pool":
                nc.vector.tensor_mul(ysq[:], y_t[:, :, ts_], y_t[:, :, ts_])
            else:  # DVE TT-mult (f16 2x) - used for batch 0 so ACT's weight
                # DMA issues don't sit ahead of the startup stats chain
                nc.vector.tensor_mul(ysq[:], y_t[:, :, ts_], y_t[:, :, ts_])
            ysq_tiles[(b, t)] = ysq

        def emit_tile_stats(b, t, sum_eng="pool"):
            """LN stats + A/B rows + broadcast + ysc for t-tile. Returns ysc.
            Chunk pre-sums cut the PE stats matmuls 8 -> 2; they run on Pool
            (DVE for the b0 prologue where Pool is conv-loaded)."""
            y_t = y_tiles[b]
            ts_ = slice(t * TN, (t + 1) * TN)
            ysq = ysq_tiles.pop((b, t))
            mu_ps = smps.tile([1, TN], F32, tag="sm")
            ey2_ps = smps.tile([1, TN], F32, tag="sm")
            for ci in range(CC):
                nc.tensor.matmul(mu_ps[:], ones_col[:], y_t[:, ci, ts_],
                                 start=(ci == 0), stop=(ci == CC - 1))
                nc.tensor.matmul(ey2_ps[:], ones_col[:], ysq[:, ci, :],
                                 start=(ci == 0), stop=(ci == CC - 1))
            mu_sb = rowp.tile([1, TN], F32, tag="r")
            nc.vector.tensor_copy(mu_sb[:], mu_ps[:])
            msq = rowp.tile([1, TN], F32, tag="r")
            nc.vector.tensor_mul(msq[:], mu_sb[:], mu_sb[:])
            var = rowp.tile([1, TN], F32, tag="r")
            nc.vector.tensor_sub(var[:], ey2_ps[:], msq[:])
            stdv = rowp.tile([1, TN], F32, tag="r")
            nc.scalar.activation(stdv[:], var[:], AF.Sqrt, bias=eps_ln[:])
            ab_row = rowp.tile([1, 2 * TN], F16, tag="ab")
            nc.vector.reciprocal(ab_row[:, 0:TN], stdv[:])
            # B = -mu * A
            nc.vector.scalar_tensor_tensor(ab_row[:, TN:2 * TN], mu_sb[:], -1.0,
                                           ab_row[:, 0:TN], OP.mult, OP.mult)
            # broadcast A,B across partitions on GPSIMD (off the PE stream;
            # the one-tile-ahead chain slack absorbs the Q7 latency)
            ab_sb = abp.tile([128, 2 * TN], F16, tag="absb")
            nc.gpsimd.partition_broadcast(ab_sb[:], ab_row[:])
            # ysc = y*A + B
            if t in FP8_TILES:
                # fp8 rhs for the DoubleRow mm1: mul to f16 temp (2x), add
                # writes f8 (1x)
                ysc = yscp.tile([128, CC, TN], F8, tag="ysc8")
                for ci in range(CC):
                    ytmp = ytp.tile([128, TN], F16, tag="yt")
                    nc.vector.tensor_mul(ytmp[:], y_t[:, ci, ts_],
                                         ab_sb[:, 0:TN])
                    nc.vector.tensor_add(ysc[:, ci, :], ytmp[:],
                                         ab_sb[:, TN:2 * TN])
                return ysc
            ysc = yscp.tile([128, CC, TN], F16, tag="ysc")
            for ci in range(CC):
                nc.vector.tensor_mul(ysc[:, ci, :], y_t[:, ci, ts_],
                                     ab_sb[:, 0:TN])
                nc.vector.tensor_add(ysc[:, ci, :], ysc[:, ci, :],
                                     ab_sb[:, TN:2 * TN])
            return ysc

        def emit_tile_mm1(b, t, ysc, mid=None, sq_dve=8):
            """mid() is invoked a few i-chunks in: the next tile's stats
            chain emitted there lands mid-PE-queue, so its matmuls don't
            block the queue head while the DVE pre-sums run."""
            gxpart = gxparts[b]
            h_t = hp.tile([128, IC, TN], F16, tag="h")
            h_tiles[(b, t)] = h_t
            mid_out = [None]
            fp8 = t in FP8_TILES
            for ii in range(IC):
                if ii == 3 and mid is not None:
                    mid_out[0] = mid()
                ph = mmps.tile([128, TN], F32, tag="mm")
                isl = slice(ii * 128, (ii + 1) * 128)
                if fp8:
                    # DoubleRow fp8: each instr contracts a 256-row chunk
                    # pair; hi then lo weight terms accumulate in PSUM
                    for wi, w8 in enumerate((w1h8, w1l8)):
                        for cp in range(CC // 2):
                            nc.tensor.matmul(
                                ph[:], w8[:, 2 * cp:2 * cp + 2, isl],
                                ysc[:, 2 * cp:2 * cp + 2, :], perf_mode=DR,
                                start=(wi == 0 and cp == 0),
                                stop=(wi == 1 and cp == CC // 2 - 1))
                    nc.scalar.activation(h_t[:, ii, :], ph[:], AF.Gelu,
                                         bias=b1p[:, ii:ii + 1],
                                         scale=1.0 / W1SCALE)
                else:
                    for ci in range(CC):
                        nc.tensor.matmul(ph[:], w1pT[:, ci, isl],
                                         ysc[:, ci, :],
                                         start=(ci == 0), stop=(ci == CC - 1))
                    nc.scalar.activation(h_t[:, ii, :], ph[:], AF.Gelu,
                                         bias=b1p[:, ii:ii + 1])
                sq = sqp.tile([128, TN], F16, tag="sq")
                # h-squares: baseline-proven split DVE-STT / ACT-Square
                if (ii in (0, 3, 5, 8, 10)) if sq_dve == 4 else (ii % 3 != 2):
                    nc.vector.scalar_tensor_tensor(
                        sq[:], h_t[:, ii, :], 1.0, h_t[:, ii, :],
                        OP.bypass, OP.mult,
                        accum_out=gxpart[:, ii, t:t + 1])
                else:
                    nc.scalar.activation(
                        sq[:], h_t[:, ii, :], AF.Square,
                        accum_out=gxpart[:, ii, t:t + 1])
            return mid_out[0]

        def emit_grn(b):
            """GRN scale factors -> scaled w2 copy."""
            gxpart = gxparts[b]
            gxsq = gxp.tile([128, IC], F32, tag="gx2")
            nc.vector.tensor_reduce(gxsq[:], gxpart[:],
                                    axis=mybir.AxisListType.X, op=OP.add)
            gx = gxp.tile([128, IC], F32R, tag="gx2")
            nc.scalar.activation(gx[:], gxsq[:], AF.Sqrt)
            gsum = smps.tile([1, IC], F32, tag="sm")
            nc.tensor.matmul(gsum[:], ones_col1[:], gx[:], start=True, stop=True)
            gtot = gxp.tile([1, 1], F32, tag="gx3")
            nc.vector.tensor_reduce(gtot[:], gsum[:], axis=mybir.AxisListType.X,
                                    op=OP.add)
            dinv = gxp.tile([1, 1], F32, tag="gx3")
            nc.vector.tensor_scalar(dinv[:], gtot[:], 1.0 / I, GRN_EPS,
                                    OP.mult, OP.add)
            d_row = gxp.tile([1, 1], F32R, tag="gx3")
            nc.vector.reciprocal(d_row[:], dinv[:])
            dbc = gxp.tile([128, 1], F32, tag="gx4")
            nc.gpsimd.partition_broadcast(dbc[:], d_row[:].bitcast(F32))
            ss = gxp.tile([128, IC], F32, tag="gx4")
            nc.vector.scalar_tensor_tensor(ss[:], gx[:].bitcast(F32), dbc[:],
                                           grng[:], OP.mult, OP.mult)
            nc.vector.tensor_scalar(ss[:], ss[:], 1.0, None, OP.add)
            w2sc = w2sp.tile([128, IC, C], F16, tag="w2sc")
            # chunk 0's columns first ([128,128] x12, ~1.1us) so mm2's
            # first po group starts before the remaining columns finish
            for ii in range(IC):
                nc.vector.tensor_scalar(w2sc[:, ii, 0:128], w2t[:, ii, 0:128],
                                        ss[:, ii:ii + 1], None, OP.mult)
            for ii in range(IC):
                nc.vector.tensor_scalar(w2sc[:, ii, 128:C], w2t[:, ii, 128:C],
                                        ss[:, ii:ii + 1], None, OP.mult)
            return w2sc

        def emit_mm2(b, w2sc, prefetch=None):
            """mm2 + bias + residual + store. prefetch() emitted mid-stream."""
            x_t = x_tiles[b]
            group = 0
            for t in range(TC):
                ts_ = slice(t * TN, (t + 1) * TN)
                h_t = h_tiles[(b, t)]
                for ci in range(CC):
                    csl = slice(ci * 128, (ci + 1) * 128)
                    po = pops.tile([128, TN], F32, tag="po")
                    for ii in range(IC):
                        nc.tensor.matmul(po[:], w2sc[:, ii, csl],
                                         h_t[:, ii, :],
                                         start=(ii == 0), stop=(ii == IC - 1))
                    o_sb = op_.tile([128, TN], F16)
                    # GPSIMD cannot read PSUM (walrus-verified): evac+bias
                    # on ACT, residual add on Pool; f16 out halves the DMA
                    nc.scalar.activation(o_sb[:], po[:], AF.Identity,
                                         bias=b2p[:, ci:ci + 1])
                    nc.gpsimd.tensor_add(o_sb[:], o_sb[:], x_t[:, ci, ts_])
                    nc.gpsimd.dma_start(
                        out_d[b, ci * 128:(ci + 1) * 128, ts_], o_sb[:])
                    group += 1
                    if group == 10 and prefetch is not None:
                        prefetch()

        # ================= schedule =================
        # stats chains run one t-tile ahead of mm1 so the LN row-math /
        # broadcast latency hides under the previous tile's matmuls; ysq
        # ops are emitted right after conv so they don't queue behind GELUs.
        CONV_SLICES = [(0, 512), (512, 1024), (1024, 2048)]
        # conv chunk split: Pool takes chunks {2,3} of batch 0 (runs at
        # kernel start / early mm1 windows when Pool is light); slice 2 is
        # emitted in two halves between stats chains so ysc(t+1) on DVE
        # isn't queued behind the whole tail of the conv.
        BN_POOL = [(3,), (3,), ()]
        prefetched_ysc = {}

        for b in range(BPC):
            if b not in y_tiles:
                y_tiles[b] = yp.tile([128, CC, T], F16, tag="y", name=f"y{b}")
                gxparts[b] = gxp.tile([128, IC, TC], F32, tag="gxpart",
                                      name=f"gxp{b}")
            if b in prefetched_ysc:
                ysc_prev = prefetched_ysc.pop(b)
                ysc_prev = emit_tile_mm1(b, 0, ysc_prev, sq_dve=4,
                                         mid=lambda b=b: emit_tile_stats(b, 1))
                # rest of this batch's conv (chunks 2/3, t2/t3 range) in
                # 512-quarters interleaved between mm1 tiles so each stats
                # chain's data lands just before its mid-hook fires
                emit_conv_slice(b, 1024, 1536, cc_list=[2, 3])
                emit_ysq(b, 2, eng="dve")
                ysc_prev = emit_tile_mm1(b, 1, ysc_prev, sq_dve=4,
                                         mid=lambda b=b: emit_tile_stats(b, 2))
                emit_conv_slice(b, 1536, 2048, cc_list=[2, 3])
                emit_ysq(b, 3, eng="dve")
                ysc_prev = emit_tile_mm1(b, 2, ysc_prev, sq_dve=4,
                                         mid=lambda b=b: emit_tile_stats(b, 3))
                emit_tile_mm1(b, 3, ysc_prev, sq_dve=4)
            else:
                emit_conv_slice(b, *CONV_SLICES[0])
                emit_ysq(b, 0, eng="dve")
                ysc0 = emit_tile_stats(b, 0, sum_eng="dve")
                emit_conv_slice(b, *CONV_SLICES[1])
                emit_ysq(b, 1, eng="dve")
                ysc1 = emit_tile_mm1(b, 0, ysc0, sq_dve=4,
                                     mid=lambda b=b: emit_tile_stats(b, 1))
                emit_conv_slice(b, 1024, 1536)
                emit_ysq(b, 2, eng="dve")
                ysc2 = emit_tile_mm1(b, 1, ysc1, sq_dve=4,
                                     mid=lambda b=b: emit_tile_stats(b, 2))
                emit_conv_slice(b, 1536, 2048)
                emit_ysq(b, 3, eng="dve")
                ysc3 = emit_tile_mm1(b, 2, ysc2, sq_dve=4,
                                     mid=lambda b=b: emit_tile_stats(b, 3))
                emit_tile_mm1(b, 3, ysc3, sq_dve=4)
            w2sc = emit_grn(b)
            prefetch = None
            if b + 1 < BPC:
                nb = b + 1
                y_tiles[nb] = yp.tile([128, CC, T], F16, tag="y", name=f"y{nb}")
                gxparts[nb] = gxp.tile([128, IC, TC], F32, tag="gxpart",
                                       name=f"gxp{nb}")
                # mm2-window conv: chunks 0-1 fully + chunk 2 first half on
                # DVE (~32us incl w2sc + prefetched stats), chunk 3 first
                # half on Pool; second halves are emitted after mm1(nb, t0).
                emit_conv_slice(nb, 0, 1024, cc_list=[0, 1])
                emit_conv_slice(nb, 1024, 2048, cc_list=[0, 1])
                emit_conv_slice(nb, 0, 1024, cc_list=[2, 3])

                def prefetch(nb=nb):
                    # ysq emitted here (mid-mm2 stream) so the ACT queue
                    # drains batch b's po groups before b+1's squares
                    emit_ysq(nb, 0)
                    prefetched_ysc[nb] = emit_tile_stats(nb, 0)
                    emit_ysq(nb, 1)
            emit_mm2(b, w2sc, prefetch=prefetch)


def _host_prep(inputs):
    w1 = inputs["w1"].astype(np.float64)
    ln_g = inputs["ln_g"].astype(np.float64)
    ln_b = inputs["ln_b"].astype(np.float64)
    w2 = inputs["w2"].astype(np.float64)
    w1p = w1 * ln_g[None, :]                         # [I, C]
    prep = {
        "b1p": (inputs["b1"].astype(np.float64) + w1 @ ln_b)
               .astype(np.float32).reshape(IC, 128).T.copy(),
        "w2T": np.ascontiguousarray(w2.T).astype(np.float16),
        "b2p": (inputs["b2"].astype(np.float64)
                + w2 @ inputs["grn_b"].astype(np.float64))
               .astype(np.float32).reshape(CC, 128).T.copy(),
        "grng": inputs["grn_g"].reshape(IC, 128).T.copy().astype(np.float32),
        "dww": inputs["dw_w"].reshape(C, K).reshape(CC, 128, K)
               .transpose(1, 0, 2).copy().astype(np.float32),
        "dwb": inputs["dw_b"].reshape(CC, 128).T.copy().astype(np.float32),
    }
    if len(FP8_TILES) < TC:
        prep["w1pT"] = np.ascontiguousarray(w1p.T).astype(np.float16)
    if FP8_TILES:
        f8np = mybir.dt.np(F8)
        w1ps = np.ascontiguousarray(w1p.T) * W1SCALE          # [C, I] f64
        w1h = w1ps.astype(np.float32).astype(f8np)
        w1l = (w1ps - w1h.astype(np.float64)).astype(np.float32).astype(f8np)
        prep["w1h8"] = w1h
        prep["w1l8"] = w1l
    return prep


def run(inputs, trace=False, **kw):
    if "nc" not in _CACHE:
        _CACHE["nc"] = _build()
    nc = _CACHE["nc"]
    prep = _host_prep(inputs)
    x = np.asarray(inputs["x"], dtype=np.float32).astype(np.float16)
    in_maps = []
    for c in range(NCORES):
        m = dict(prep)
        m["x"] = np.ascontiguousarray(x[c * BPC:(c + 1) * BPC])
        in_maps.append(m)
    res = run_bass_kernel_spmd(nc, in_maps, core_ids=list(range(NCORES)),
                               trace=trace, **kw)
    out = np.concatenate([r["out"] for r in res.results], axis=0)
    return out.astype(np.float32), res


def kernel(**inputs):
    out, _ = run(inputs)
    return out

